# revision 32
# baseline (speedup 1.0000x reference)
"""Class-based decoder (MoE-style routing) on 8 trn2 NeuronCores.

Strategy: expert-parallel. Classes are padded 250->256 and split 32 per core.
On the host, tokens are grouped by class into capacity-padded slots (C tokens
per class slot, C in {32,64,128}); class slots that overflow C spill into
extra slots holding a duplicate of the class weights.  Each core receives:
  - xT   [128, n_mt*KCH*128]   its padded tokens, pre-transposed k-major
  - wcT  [128, KCH*NCLS_P]     the (replicated) class-decoder weights, k-major
  - wwT  [n_grp, 128, GRP*KCH*CHUNK]  its word-decoder shard, k-major, grouped
and computes, for every 128-token tile, the class logits (x @ Wc.T) and the
per-class word logits (x_c @ Ww[c].T) as PE matmuls accumulating K=512 over
4 PSUM chunks.  Class slots of a tile are col-tiled into one PSUM tile.
Biases (zero in practice, but handled for correctness) are added on the host
during the final unpermute.

Precision modes:
  f32  : exact fp32 matmuls (2-pass LOW/HIGH on PE; bit-exact, slowest)
  f32r : fp32 data, single-pass PE (TF32-like rounding). Classes are paired
         into N=400 matmuls and NCLS padded to 256 so the free dim is >=256,
         where f32r streams at full rate.
  bf16 : weights/activations cast to bf16 (halves the big W DMA)
"""

import numpy as np
from contextlib import ExitStack

import concourse.bass as bass
import concourse.bacc as bacc
import concourse.tile as tile
import concourse.mybir as mybir
from concourse.bass_utils import run_bass_kernel_spmd

NHID = 512
NCLS = 250
CHUNK = 200
NCORES = 8
KCH = NHID // 128          # 4 contraction chunks of 128
NCLS_PAD = 256             # classes padded so each core owns an equal shard
CPC = NCLS_PAD // NCORES   # classes per core
NCOL = NCLS + CHUNK        # 450 output columns
F32 = mybir.dt.float32

MODE = "bf16"              # default precision mode; see module docstring

LAST_RESULT = None         # BassKernelResults of the most recent device run
_program_cache = {}

_MM_DT = {"f32": mybir.dt.float32, "f32r": mybir.dt.float32r,
          "bf16": mybir.dt.bfloat16}
_NP_DT = {"f32": np.float32, "f32r": np.float32, "bf16": None}  # bf16 set below
try:
    import ml_dtypes
    _NP_DT["bf16"] = ml_dtypes.bfloat16
except ImportError:
    pass


def _build_program(C, slots, mode):
    """One SPMD program: slots class-slots of C tokens each, per core.

    f32 uses the "coltile" scheme: per class slot, an M=C matmul col-tiled
    into a shared PSUM tile (exact 2-pass fp32).
    f32r/bf16 use the "block" scheme: every matmul is M=128 (all slots of an
    m-tile), and the word logits come as per_mt//2 halves of N=2*CHUNK whose
    off-diagonal class blocks are discarded by the PSUM->SBUF copies.  This
    keeps N>=256 (full-rate f32r) and NumWeights=128 (FWL weight loads).
    """
    n_mt = (slots * C) // 128  # 128-token m-tiles
    npad = slots * C
    per_mt = 128 // C          # class slots per m-tile
    block = mode in ("f32r", "bf16")
    # class slots per pw matmul and word-columns per pw matmul
    gs = 2 if (block and per_mt >= 2) else 1
    gw = gs * CHUNK            # 400 paired / 200 single
    n_half = per_mt // gs      # pw matmul groups per m-tile
    ncls_p = 256 if block else NCLS  # N>=256 keeps f32r at full rate
    # C=16 diag copies would need 16-partition bases (illegal); store each
    # 32-row band's full pair block instead and let the host pick the diagonal
    wide = block and C == 16
    ocol = NCLS + (gw if wide else CHUNK)
    dt = _MM_DT[mode]

    nc = bacc.Bacc("TRN2", target_bir_lowering=False, debug=False,
                   num_devices=NCORES)
    xT = nc.dram_tensor("xT", [128, n_mt * KCH * 128], dt, kind="ExternalInput")
    wcT = nc.dram_tensor("wcT", [128, KCH * ncls_p], dt, kind="ExternalInput")
    # W groups: one DMA per m-tile worth of class slots
    wwT = nc.dram_tensor("wwT", [n_mt, 128, per_mt * KCH * CHUNK], dt,
                         kind="ExternalInput")
    out = nc.dram_tensor("out", [npad, ocol], F32, kind="ExternalOutput")

    with tile.TileContext(nc) as tc, ExitStack() as ctx:
        xpool = ctx.enter_context(tc.tile_pool(name="x", bufs=1))
        wcpool = ctx.enter_context(tc.tile_pool(name="wc", bufs=1))
        wpool = ctx.enter_context(tc.tile_pool(name="w", bufs=12))
        opool = ctx.enter_context(tc.tile_pool(name="o", bufs=8))
        pcp = ctx.enter_context(
            tc.tile_pool(name="pc", bufs=2, space=bass.MemorySpace.PSUM))
        pwp = ctx.enter_context(
            tc.tile_pool(name="pw", bufs=6, space=bass.MemorySpace.PSUM))

        # three independent DMA streams so nothing blocks the W firehose:
        #   sync (SP HWDGE): only the big W chunks, back to back
        #   scalar (ACT HWDGE): wc + per-m-tile x loads
        #   gpsimd (SWDGE): output stores
        wc_sb = wcpool.tile([128, KCH * ncls_p], dt)
        x_sb = xpool.tile([128, n_mt * KCH * 128], dt)

        wchunk = KCH * gw  # free-dim elems per W DMA (one pw matmul group)
        for m in range(n_mt):
            # x columns for this m-tile: [(m*KCH+j)*128 + t]
            nc.scalar.dma_start(x_sb[:, m * KCH * 128:(m + 1) * KCH * 128],
                                xT[:, m * KCH * 128:(m + 1) * KCH * 128])
            if m == 0:
                nc.scalar.dma_start(wc_sb[:], wcT[:])

            def xcol(j, lo, hi):
                base = (m * KCH + j) * 128
                return x_sb[:, base + lo:base + hi]

            o_sb = opool.tile([128, ocol], F32)

            if block:
                # word logits: per half, one M=128 matmul of N=gw covering
                # gs classes; only each slot's own class block is kept
                for h in range(n_half):
                    w_sb = wpool.tile([128, wchunk], dt, tag="w")
                    weng = nc.sync if (m * n_half + h) % 2 == 0 else nc.scalar
                    weng.dma_start(
                        w_sb[:], wwT[m][:, h * wchunk:(h + 1) * wchunk])
                    pw_ps = pwp.tile([128, gw], F32, tag="pw")
                    for j in range(KCH):
                        nc.tensor.matmul(
                            pw_ps[:, :],
                            xcol(j, 0, 128),
                            w_sb[:, j * gw:(j + 1) * gw],
                            start=(j == 0), stop=(j == KCH - 1),
                        )
                    if wide:
                        b = gs * C  # 32-row band of this pair
                        nc.vector.tensor_copy(
                            o_sb[h * b:(h + 1) * b, NCLS:],
                            pw_ps[h * b:(h + 1) * b, :])
                    else:
                        for a in range(gs):
                            q = h * gs + a  # slot in m-tile
                            nc.vector.tensor_copy(
                                o_sb[q * C:(q + 1) * C, NCLS:],
                                pw_ps[q * C:(q + 1) * C,
                                      a * CHUNK:(a + 1) * CHUNK])
            else:
                # exact f32: per-slot M=C matmuls col-tiled into one tile
                w_sb = wpool.tile([128, per_mt * KCH * CHUNK], dt, tag="w")
                nc.sync.dma_start(w_sb[:], wwT[m])
                pw_ps = pwp.tile([128, CHUNK], F32, tag="pw")
                for q in range(per_mt):
                    for j in range(KCH):
                        nc.tensor.matmul(
                            pw_ps[q * C:(q + 1) * C, :],
                            xcol(j, q * C, (q + 1) * C),
                            w_sb[:, (q * KCH + j) * CHUNK:
                                 (q * KCH + j + 1) * CHUNK],
                            start=(j == 0), stop=(j == KCH - 1),
                            tile_position=(0, q * C),
                        )
                nc.vector.tensor_copy(o_sb[:, NCLS:], pw_ps[:])

            # class logits last: wc arrives behind the first W chunks
            pc_ps = pcp.tile([128, ncls_p], F32)
            for j in range(KCH):
                nc.tensor.matmul(
                    pc_ps[:, :],
                    xcol(j, 0, 128),
                    wc_sb[:, j * ncls_p:(j + 1) * ncls_p],
                    start=(j == 0), stop=(j == KCH - 1),
                )
            nc.vector.tensor_copy(o_sb[:, :NCLS], pc_ps[:, :NCLS])

            nc.gpsimd.dma_start(out[m * 128:(m + 1) * 128, :], o_sb[:])

    nc.compile()
    return nc


def _route(cls, mode):
    """Group tokens by class into capacity-padded slots: one slot per class,
    C tokens of capacity.  The (rare) tokens beyond a class's capacity are
    returned as `overflow` and evaluated directly on the host in numpy.

    Returns (C, slots, tok_idx [NCORES, slots*C] int64 token id or -1,
    slot_cls [NCORES, slots] class id per slot, overflow token-id array).
    """
    counts = np.bincount(cls, minlength=NCLS_PAD)
    # coltile (exact f32) needs C to be a multiple of 32 for PSUM col tiling
    cands = (16, 32, 64, 128) if mode in ("f32r", "bf16") else (32, 64, 128)
    C = cands[-1]
    for c in cands:
        if int(np.maximum(counts - c, 0).sum()) <= 32:
            C = c
            break

    order = np.argsort(cls, kind="stable")
    starts = np.zeros(NCLS_PAD + 1, np.int64)
    starts[1:] = np.cumsum(counts)

    slots = CPC  # one slot per class owned by the core
    tok_idx = np.full((NCORES, slots * C), -1, np.int64)
    slot_cls = np.full((NCORES, slots), -1, np.int64)
    overflow = []
    for k in range(NCORES):
        for s in range(slots):
            c = k * CPC + s
            lo, cnt = int(starts[c]), int(counts[c])
            n = min(C, cnt)
            slot_cls[k, s] = c
            if n > 0:
                tok_idx[k, s * C:s * C + n] = order[lo:lo + n]
            if cnt > C:
                overflow.append(order[lo + C:lo + cnt])
    overflow = (np.concatenate(overflow) if overflow
                else np.zeros((0,), np.int64))
    return C, slots, tok_idx, slot_cls, overflow


def kernel(x, Wc, bc, Ww, bw, cls_idx, _trace=False, _trace_cores=None,
           _mode=None):
    global LAST_RESULT
    mode = _mode or MODE
    ndt = _NP_DT[mode]
    if ndt is None:
        mode = "f32"
        ndt = np.float32

    x = np.ascontiguousarray(np.asarray(x, np.float32))
    Wc = np.ascontiguousarray(np.asarray(Wc, np.float32))
    bc = np.asarray(bc, np.float32)
    Ww = np.ascontiguousarray(np.asarray(Ww, np.float32))
    bw = np.asarray(bw, np.float32)
    cls = np.asarray(cls_idx).astype(np.int64).ravel()
    N = cls.shape[0]

    C, slots, tok_idx, slot_cls, overflow = _route(cls, mode)
    npad = slots * C
    n_mt = npad // 128
    per_mt = 128 // C
    block = mode in ("f32r", "bf16")
    gs = 2 if (block and per_mt >= 2) else 1
    ncls_p = 256 if block else NCLS

    key = (C, slots, mode)
    if key not in _program_cache:
        _program_cache[key] = _build_program(C, slots, mode)
    nc = _program_cache[key]

    # wcT [128, KCH*ncls_p]: wcT[p, j*ncls_p+c] = Wc[c, j*128+p]  (replicated)
    Wc_p = Wc if ncls_p == NCLS else np.concatenate(
        [Wc, np.zeros((ncls_p - NCLS, NHID), np.float32)], 0)
    wcT = np.ascontiguousarray(
        Wc_p.reshape(ncls_p, KCH, 128).transpose(2, 1, 0)
            .reshape(128, KCH * ncls_p).astype(ndt))

    Ww_pad = np.zeros((NCLS_PAD, CHUNK, NHID), np.float32)
    Ww_pad[:NCLS] = Ww

    in_maps = []
    for k in range(NCORES):
        # per-slot k-major weights: tmp[s, j, p, w] = Ww[cls_s, w, j*128+p]
        wsel = Ww_pad[np.maximum(slot_cls[k], 0)]
        wsel[slot_cls[k] < 0] = 0.0
        tmp = wsel.reshape(slots, CHUNK, KCH, 128).transpose(0, 2, 3, 1)
        if gs == 2:
            # group = m-tile (per_mt slots); within: pair r, then j, then
            # the two slots' CHUNK columns side by side
            tmp = tmp.reshape(n_mt, per_mt // 2, 2, KCH, 128, CHUNK)
            tmp = tmp.transpose(0, 4, 1, 3, 2, 5)  # [n_mt,128,pair,j,2,CHUNK]
        else:
            tmp = tmp.reshape(n_mt, per_mt, KCH, 128, CHUNK)
            tmp = tmp.transpose(0, 3, 1, 2, 4)     # [n_mt,128,q,j,CHUNK]
        wwT = np.ascontiguousarray(
            tmp.reshape(n_mt, 128, per_mt * KCH * CHUNK).astype(ndt))

        ti = tok_idx[k]
        xk = x[np.maximum(ti, 0)]
        xk[ti < 0] = 0.0
        # xT[p, (m*KCH+j)*128 + t] = xk[m*128+t, j*128+p]
        xT = np.ascontiguousarray(
            xk.reshape(n_mt, 128, KCH, 128).transpose(3, 0, 2, 1)
              .reshape(128, n_mt * KCH * 128).astype(ndt))
        in_maps.append({"xT": xT, "wcT": wcT, "wwT": wwT})

    LAST_RESULT = run_bass_kernel_spmd(
        nc, in_maps, list(range(NCORES)), trace=_trace,
        trace_cores=(_trace_cores if _trace else None))

    wide = block and C == 16
    out = np.zeros((N, NCOL), np.float32)
    if wide:
        # row r of a core's output holds its pair's full 2*CHUNK block;
        # slot parity selects which CHUNK half is this row's class
        a_row = (np.arange(npad) // C) % 2
    for k in range(NCORES):
        ok = np.asarray(LAST_RESULT.results[k]["out"], np.float32)
        if wide:
            words = np.where((a_row == 0)[:, None],
                             ok[:, NCLS:NCLS + CHUNK],
                             ok[:, NCLS + CHUNK:NCLS + 2 * CHUNK])
            ok = np.concatenate([ok[:, :NCLS], words], 1)
        valid = tok_idx[k] >= 0
        out[tok_idx[k][valid]] = ok[valid]

    if overflow.size:
        # rare capacity-overflow tokens: evaluate directly on the host
        xo = x[overflow]                                   # [no, NHID]
        out[overflow, :NCLS] = xo @ Wc.T
        co = cls[overflow]
        out[overflow, NCLS:] = np.einsum(
            "nkh,nh->nk", Ww[co], xo, optimize=True)

    out[:, :NCLS] += bc
    out[:, NCLS:] += bw[cls]
    return out


# revision 33
# speedup vs baseline: 1.1644x; 1.1644x over previous
"""Class-based decoder (MoE-style routing) on 8 trn2 NeuronCores.

Strategy: expert-parallel. Classes are padded 250->256 and split 32 per core.
On the host, tokens are grouped by class into capacity-padded slots (C tokens
per class slot, C in {32,64,128}); class slots that overflow C spill into
extra slots holding a duplicate of the class weights.  Each core receives:
  - xT   [128, n_mt*KCH*128]   its padded tokens, pre-transposed k-major
  - wcT  [128, KCH*NCLS_P]     the (replicated) class-decoder weights, k-major
  - wwT  [n_grp, 128, GRP*KCH*CHUNK]  its word-decoder shard, k-major, grouped
and computes, for every 128-token tile, the class logits (x @ Wc.T) and the
per-class word logits (x_c @ Ww[c].T) as PE matmuls accumulating K=512 over
4 PSUM chunks.  Class slots of a tile are col-tiled into one PSUM tile.
Biases (zero in practice, but handled for correctness) are added on the host
during the final unpermute.

Precision modes:
  f32  : exact fp32 matmuls (2-pass LOW/HIGH on PE; bit-exact, slowest)
  f32r : fp32 data, single-pass PE (TF32-like rounding). Classes are paired
         into N=400 matmuls and NCLS padded to 256 so the free dim is >=256,
         where f32r streams at full rate.
  bf16 : weights/activations cast to bf16 (halves the big W DMA)
"""

import numpy as np
from contextlib import ExitStack

import concourse.bass as bass
import concourse.bacc as bacc
import concourse.tile as tile
import concourse.mybir as mybir
from concourse.bass_utils import run_bass_kernel_spmd

NHID = 512
NCLS = 250
CHUNK = 200
NCORES = 8
KCH = NHID // 128          # 4 contraction chunks of 128
NCLS_PAD = 256             # classes padded so each core owns an equal shard
CPC = NCLS_PAD // NCORES   # classes per core
NCOL = NCLS + CHUNK        # 450 output columns
F32 = mybir.dt.float32

MODE = "bf16"              # default precision mode; see module docstring

LAST_RESULT = None         # BassKernelResults of the most recent device run
_program_cache = {}

_MM_DT = {"f32": mybir.dt.float32, "f32r": mybir.dt.float32r,
          "bf16": mybir.dt.bfloat16}
_NP_DT = {"f32": np.float32, "f32r": np.float32, "bf16": None}  # bf16 set below
try:
    import ml_dtypes
    _NP_DT["bf16"] = ml_dtypes.bfloat16
except ImportError:
    pass


def _build_program(C, slots, mode):
    """One SPMD program: slots class-slots of C tokens each, per core.

    f32 uses the "coltile" scheme: per class slot, an M=C matmul col-tiled
    into a shared PSUM tile (exact 2-pass fp32).
    f32r/bf16 use the "block" scheme: every matmul is M=128 (all slots of an
    m-tile), and the word logits come as per_mt//2 halves of N=2*CHUNK whose
    off-diagonal class blocks are discarded by the PSUM->SBUF copies.  This
    keeps N>=256 (full-rate f32r) and NumWeights=128 (FWL weight loads).
    """
    n_mt = (slots * C) // 128  # 128-token m-tiles
    npad = slots * C
    per_mt = 128 // C          # class slots per m-tile
    block = mode in ("f32r", "bf16")
    # class slots per pw matmul and word-columns per pw matmul
    gs = 2 if (block and per_mt >= 2) else 1
    gw = gs * CHUNK            # 400 paired / 200 single
    n_half = per_mt // gs      # pw matmul groups per m-tile
    ncls_p = 256 if block else NCLS  # N>=256 keeps f32r at full rate
    # C=16 diag copies would need 16-partition bases (illegal); store each
    # 32-row band's full pair block instead and let the host pick the diagonal
    wide = block and C == 16
    ocol = NCLS + (gw if wide else CHUNK)
    dt = _MM_DT[mode]

    nc = bacc.Bacc("TRN2", target_bir_lowering=False, debug=False,
                   num_devices=NCORES)
    xT = nc.dram_tensor("xT", [128, n_mt * KCH * 128], dt, kind="ExternalInput")
    wcT = nc.dram_tensor("wcT", [128, KCH * ncls_p], dt, kind="ExternalInput")
    # W groups: one DMA per m-tile worth of class slots
    wwT = nc.dram_tensor("wwT", [n_mt, 128, per_mt * KCH * CHUNK], dt,
                         kind="ExternalInput")
    out = nc.dram_tensor("out", [npad, ocol], F32, kind="ExternalOutput")

    with tile.TileContext(nc) as tc, ExitStack() as ctx:
        xpool = ctx.enter_context(tc.tile_pool(name="x", bufs=1))
        wcpool = ctx.enter_context(tc.tile_pool(name="wc", bufs=1))
        wpool = ctx.enter_context(tc.tile_pool(name="w", bufs=12))
        opool = ctx.enter_context(tc.tile_pool(name="o", bufs=8))
        pcp = ctx.enter_context(
            tc.tile_pool(name="pc", bufs=2, space=bass.MemorySpace.PSUM))
        pwp = ctx.enter_context(
            tc.tile_pool(name="pw", bufs=6, space=bass.MemorySpace.PSUM))

        # three independent DMA streams so nothing blocks the W firehose:
        #   sync (SP HWDGE): only the big W chunks, back to back
        #   scalar (ACT HWDGE): wc + per-m-tile x loads
        #   gpsimd (SWDGE): output stores
        wc_sb = wcpool.tile([128, KCH * ncls_p], dt)
        nc.scalar.dma_start(wc_sb[:], wcT[:])
        x_sb = xpool.tile([128, n_mt * KCH * 128], dt)

        wchunk = KCH * gw  # free-dim elems per W DMA (one pw matmul group)
        for m in range(n_mt):
            # x columns for this m-tile: [(m*KCH+j)*128 + t]
            nc.scalar.dma_start(x_sb[:, m * KCH * 128:(m + 1) * KCH * 128],
                                xT[:, m * KCH * 128:(m + 1) * KCH * 128])

            def xcol(j, lo, hi):
                base = (m * KCH + j) * 128
                return x_sb[:, base + lo:base + hi]

            # class logits for these 128 tokens
            pc_ps = pcp.tile([128, ncls_p], F32)
            for j in range(KCH):
                nc.tensor.matmul(
                    pc_ps[:, :],
                    xcol(j, 0, 128),
                    wc_sb[:, j * ncls_p:(j + 1) * ncls_p],
                    start=(j == 0), stop=(j == KCH - 1),
                )

            o_sb = opool.tile([128, ocol], F32)
            nc.vector.tensor_copy(o_sb[:, :NCLS], pc_ps[:, :NCLS])

            if block:
                # word logits: per half, one M=128 matmul of N=gw covering
                # gs classes; only each slot's own class block is kept
                for h in range(n_half):
                    w_sb = wpool.tile([128, wchunk], dt, tag="w")
                    weng = nc.sync if (m * n_half + h) % 2 == 0 else nc.scalar
                    weng.dma_start(
                        w_sb[:], wwT[m][:, h * wchunk:(h + 1) * wchunk])
                    pw_ps = pwp.tile([128, gw], F32, tag="pw")
                    for j in range(KCH):
                        nc.tensor.matmul(
                            pw_ps[:, :],
                            xcol(j, 0, 128),
                            w_sb[:, j * gw:(j + 1) * gw],
                            start=(j == 0), stop=(j == KCH - 1),
                        )
                    if wide:
                        b = gs * C  # 32-row band of this pair
                        nc.vector.tensor_copy(
                            o_sb[h * b:(h + 1) * b, NCLS:],
                            pw_ps[h * b:(h + 1) * b, :])
                    else:
                        for a in range(gs):
                            q = h * gs + a  # slot in m-tile
                            nc.vector.tensor_copy(
                                o_sb[q * C:(q + 1) * C, NCLS:],
                                pw_ps[q * C:(q + 1) * C,
                                      a * CHUNK:(a + 1) * CHUNK])
            else:
                # exact f32: per-slot M=C matmuls col-tiled into one tile
                w_sb = wpool.tile([128, per_mt * KCH * CHUNK], dt, tag="w")
                nc.sync.dma_start(w_sb[:], wwT[m])
                pw_ps = pwp.tile([128, CHUNK], F32, tag="pw")
                for q in range(per_mt):
                    for j in range(KCH):
                        nc.tensor.matmul(
                            pw_ps[q * C:(q + 1) * C, :],
                            xcol(j, q * C, (q + 1) * C),
                            w_sb[:, (q * KCH + j) * CHUNK:
                                 (q * KCH + j + 1) * CHUNK],
                            start=(j == 0), stop=(j == KCH - 1),
                            tile_position=(0, q * C),
                        )
                nc.vector.tensor_copy(o_sb[:, NCLS:], pw_ps[:])

            nc.gpsimd.dma_start(out[m * 128:(m + 1) * 128, :], o_sb[:])

    nc.compile()
    return nc


def _route(cls, mode):
    """Group tokens by class into capacity-padded slots: one slot per class,
    C tokens of capacity.  The (rare) tokens beyond a class's capacity are
    returned as `overflow` and evaluated directly on the host in numpy.

    Returns (C, slots, tok_idx [NCORES, slots*C] int64 token id or -1,
    slot_cls [NCORES, slots] class id per slot, overflow token-id array).
    """
    counts = np.bincount(cls, minlength=NCLS_PAD)
    # coltile (exact f32) needs C to be a multiple of 32 for PSUM col tiling
    cands = (16, 32, 64, 128) if mode in ("f32r", "bf16") else (32, 64, 128)
    C = cands[-1]
    for c in cands:
        if int(np.maximum(counts - c, 0).sum()) <= 32:
            C = c
            break

    order = np.argsort(cls, kind="stable")
    starts = np.zeros(NCLS_PAD + 1, np.int64)
    starts[1:] = np.cumsum(counts)

    slots = CPC  # one slot per class owned by the core
    tok_idx = np.full((NCORES, slots * C), -1, np.int64)
    slot_cls = np.full((NCORES, slots), -1, np.int64)
    overflow = []
    for k in range(NCORES):
        for s in range(slots):
            c = k * CPC + s
            lo, cnt = int(starts[c]), int(counts[c])
            n = min(C, cnt)
            slot_cls[k, s] = c
            if n > 0:
                tok_idx[k, s * C:s * C + n] = order[lo:lo + n]
            if cnt > C:
                overflow.append(order[lo + C:lo + cnt])
    overflow = (np.concatenate(overflow) if overflow
                else np.zeros((0,), np.int64))
    return C, slots, tok_idx, slot_cls, overflow


def kernel(x, Wc, bc, Ww, bw, cls_idx, _trace=False, _trace_cores=None,
           _mode=None):
    global LAST_RESULT
    mode = _mode or MODE
    ndt = _NP_DT[mode]
    if ndt is None:
        mode = "f32"
        ndt = np.float32

    x = np.ascontiguousarray(np.asarray(x, np.float32))
    Wc = np.ascontiguousarray(np.asarray(Wc, np.float32))
    bc = np.asarray(bc, np.float32)
    Ww = np.ascontiguousarray(np.asarray(Ww, np.float32))
    bw = np.asarray(bw, np.float32)
    cls = np.asarray(cls_idx).astype(np.int64).ravel()
    N = cls.shape[0]

    C, slots, tok_idx, slot_cls, overflow = _route(cls, mode)
    npad = slots * C
    n_mt = npad // 128
    per_mt = 128 // C
    block = mode in ("f32r", "bf16")
    gs = 2 if (block and per_mt >= 2) else 1
    ncls_p = 256 if block else NCLS

    key = (C, slots, mode)
    if key not in _program_cache:
        _program_cache[key] = _build_program(C, slots, mode)
    nc = _program_cache[key]

    # wcT [128, KCH*ncls_p]: wcT[p, j*ncls_p+c] = Wc[c, j*128+p]  (replicated)
    Wc_p = Wc if ncls_p == NCLS else np.concatenate(
        [Wc, np.zeros((ncls_p - NCLS, NHID), np.float32)], 0)
    wcT = np.ascontiguousarray(
        Wc_p.reshape(ncls_p, KCH, 128).transpose(2, 1, 0)
            .reshape(128, KCH * ncls_p).astype(ndt))

    Ww_pad = np.zeros((NCLS_PAD, CHUNK, NHID), np.float32)
    Ww_pad[:NCLS] = Ww

    in_maps = []
    for k in range(NCORES):
        # per-slot k-major weights: tmp[s, j, p, w] = Ww[cls_s, w, j*128+p]
        wsel = Ww_pad[np.maximum(slot_cls[k], 0)]
        wsel[slot_cls[k] < 0] = 0.0
        tmp = wsel.reshape(slots, CHUNK, KCH, 128).transpose(0, 2, 3, 1)
        if gs == 2:
            # group = m-tile (per_mt slots); within: pair r, then j, then
            # the two slots' CHUNK columns side by side
            tmp = tmp.reshape(n_mt, per_mt // 2, 2, KCH, 128, CHUNK)
            tmp = tmp.transpose(0, 4, 1, 3, 2, 5)  # [n_mt,128,pair,j,2,CHUNK]
        else:
            tmp = tmp.reshape(n_mt, per_mt, KCH, 128, CHUNK)
            tmp = tmp.transpose(0, 3, 1, 2, 4)     # [n_mt,128,q,j,CHUNK]
        wwT = np.ascontiguousarray(
            tmp.reshape(n_mt, 128, per_mt * KCH * CHUNK).astype(ndt))

        ti = tok_idx[k]
        xk = x[np.maximum(ti, 0)]
        xk[ti < 0] = 0.0
        # xT[p, (m*KCH+j)*128 + t] = xk[m*128+t, j*128+p]
        xT = np.ascontiguousarray(
            xk.reshape(n_mt, 128, KCH, 128).transpose(3, 0, 2, 1)
              .reshape(128, n_mt * KCH * 128).astype(ndt))
        in_maps.append({"xT": xT, "wcT": wcT, "wwT": wwT})

    LAST_RESULT = run_bass_kernel_spmd(
        nc, in_maps, list(range(NCORES)), trace=_trace,
        trace_cores=(_trace_cores if _trace else None))

    wide = block and C == 16
    out = np.zeros((N, NCOL), np.float32)
    if wide:
        # row r of a core's output holds its pair's full 2*CHUNK block;
        # slot parity selects which CHUNK half is this row's class
        a_row = (np.arange(npad) // C) % 2
    for k in range(NCORES):
        ok = np.asarray(LAST_RESULT.results[k]["out"], np.float32)
        if wide:
            words = np.where((a_row == 0)[:, None],
                             ok[:, NCLS:NCLS + CHUNK],
                             ok[:, NCLS + CHUNK:NCLS + 2 * CHUNK])
            ok = np.concatenate([ok[:, :NCLS], words], 1)
        valid = tok_idx[k] >= 0
        out[tok_idx[k][valid]] = ok[valid]

    if overflow.size:
        # rare capacity-overflow tokens: evaluate directly on the host
        xo = x[overflow]                                   # [no, NHID]
        out[overflow, :NCLS] = xo @ Wc.T
        co = cls[overflow]
        out[overflow, NCLS:] = np.einsum(
            "nkh,nh->nk", Ww[co], xo, optimize=True)

    out[:, :NCLS] += bc
    out[:, NCLS:] += bw[cls]
    return out
